# revision 12
# baseline (speedup 1.0000x reference)
"""Trainium2 Bass kernel for nn_DSF_GPR_I (gnn_message_passing), 8 NeuronCores.

Strategy (per sharding_hint: shard nodes across devices; N x N sigmoid
correlation row-block-parallel with all-gathered state):
  - 6000 nodes -> 8 ranks x 750, padded to 768 (=6*128) per rank.
  - gcn_norm scatter-add propagation A@v is computed as a dense matmul against
    the per-rank column block of the adjacency COUNT matrix (integer edge
    multiplicities, exact in bf16, SBUF-resident), with D^-1/2 applied on
    device: source side folded into the all-gathered vectors, target side via
    a broadcast multiply.
  - Per iteration: one AllGather (bf16) of [x_s | pe_s | pe_raw] (node-major)
    + qT (feature-major); prop + the row-block S = sigmoid(q_blk @ q^T) tiles
    (48 chunks of 128 columns) + corr accumulation all run from SBUF.
  - Host does integer-only index preprocessing (bincount degrees, dense count
    matrix, transposes/padding). All float math runs on device.

Partition layout: x-state on partitions 0:64, pe-state on partitions 64:128
(DVE/ACT lanes are partition-hardwired, so elementwise chains stay on a
consistent base; stationary matmul operands that pair with pe-state are loaded
at base 64).
"""
import os
import numpy as np
import ml_dtypes

R = 8
NREAL = 750
NPAD = 768
NG = R * NPAD           # 6144
NCH = NG // 128         # 48 chunks
BCH = NPAD // 128       # 6 chunks per rank block
K = 10
VA = NPAD * 128         # 98304  (node-major region: [x_raw | -0.25*pe])
QTW = 64 * NPAD         # 49152   (feature-major qT region)
AGW = VA + QTW          # 196608

LAST_EXEC_NS = None
_NC = None


def _build():
    import concourse.bacc as bacc
    import concourse.mybir as mybir
    import concourse.tile as tile

    F32 = mybir.dt.float32
    BF16 = mybir.dt.bfloat16
    AF = mybir.ActivationFunctionType

    nc = bacc.Bacc("TRN2", target_bir_lowering=False, debug=False, num_devices=R)

    # ---- I/O ----
    p_nfT = nc.declare_dram_parameter("nfT", [512, NPAD], F32, isOutput=False)
    p_posT = nc.declare_dram_parameter("posT", [32, NPAD], F32, isOutput=False)
    p_cnt = nc.declare_dram_parameter("cnt", [NG, NPAD], BF16, isOutput=False)
    p_deg = nc.declare_dram_parameter("deg", [1, NPAD], F32, isOutput=False)
    p_degall = nc.declare_dram_parameter("deg_all", [1, NG], F32, isOutput=False)
    p_W1 = nc.declare_dram_parameter("W1", [512, 256], F32, isOutput=False)
    p_W2 = nc.declare_dram_parameter("W2", [256, 64], F32, isOutput=False)
    p_Wpe = nc.declare_dram_parameter("Wpe", [32, 128], F32, isOutput=False)  # [0|Wpe]
    p_Wc = nc.declare_dram_parameter("Wc", [64, 64], F32, isOutput=False)
    p_b1 = nc.declare_dram_parameter("b1", [128, 2], F32, isOutput=False)
    p_bxp = nc.declare_dram_parameter("bias_xpe", [128, 1], F32, isOutput=False)  # [b2; bpe]
    p_bc = nc.declare_dram_parameter("bc", [64, 1], F32, isOutput=False)
    p_cwT = nc.declare_dram_parameter("cwT", [64, K + 1], F32, isOutput=False)
    p_cb = nc.declare_dram_parameter("cb", [1, K + 1], F32, isOutput=False)
    p_tmb = nc.declare_dram_parameter("tmb", [1, (K + 1) * 64], F32, isOutput=False)
    p_id = nc.declare_dram_parameter("ident", [128, 128], F32, isOutput=False)
    p_ones = nc.declare_dram_parameter("ones128", [1, 128], F32, isOutput=False)
    out_h = nc.declare_dram_parameter("out_h", [64, NPAD], F32, isOutput=True)
    out_pe = nc.declare_dram_parameter("out_pe", [64, NPAD], F32, isOutput=True)

    with tile.TileContext(nc) as tc:
        with (
            tc.tile_pool(name="pers", bufs=1) as pers,
            tc.tile_pool(name="vstp", bufs=1) as vstp,
            tc.tile_pool(name="qtfp", bufs=1) as qtfp,
            tc.tile_pool(name="spool", bufs=2) as spool,
            tc.tile_pool(name="stpool", bufs=2) as stpool,
            tc.tile_pool(name="work", bufs=2) as work,
            tc.tile_pool(name="dramp", bufs=2, space="DRAM") as dramp,
            tc.tile_pool(name="ps_a", bufs=1, space="PSUM") as ps_a,
            tc.tile_pool(name="ps_s", bufs=1, space="PSUM") as ps_s,
            tc.tile_pool(name="ps_c", bufs=1, space="PSUM") as ps_c,
        ):
            # ---- persistent SBUF ----
            A_sb = pers.tile([128, NCH, NPAD], BF16)
            vT = pers.tile([128, NPAD], F32)     # rows 0:64 x, 64:128 pe
            hr = pers.tile([128, NPAD], F32)     # rows 0:64 hidden, 64:128 0.5*pe0
            qT_blk = pers.tile([128, NPAD], BF16)
            dbt = pers.tile([128, NPAD], F32)    # rows 0:64 dinv, 64:128 0.75*dinv
            dinv_pp = pers.tile([128, BCH], F32)
            dinv_all = pers.tile([128, NCH], F32)
            W1_sb = pers.tile([128, 4, 256], F32)
            W2_sb = pers.tile([128, 2, 64], F32)
            Wpe_sb = pers.tile([32, 128], F32)
            Wc_sb = pers.tile([128, 64], F32)    # rows 64:128 hold Wc
            b1_sb = pers.tile([128, 2], F32)
            bxp_sb = pers.tile([128, 1], F32)
            bc_sb = pers.tile([64, 1], F32)
            cwT_sb = pers.tile([128, K + 1], F32)  # rows 64:128 hold coeff_w^T
            cb_sb = pers.tile([1, K + 1], F32)
            tmb_sb = pers.tile([1, (K + 1) * 64], F32)
            id_sb = pers.tile([128, 128], F32)
            ones_sb = pers.tile([1, 128], F32)
            posT_sb = pers.tile([32, NPAD], F32)
            h1T = pers.tile([128, NPAD], F32)
            dinvrow = pers.tile([1, NPAD], F32)

            # ---- input loads ----
            nc.sync.dma_start(A_sb[:], p_cnt[:].rearrange("(c p) t -> p c t", p=128))
            nc.sync.dma_start(posT_sb[:], p_posT[:])
            nc.sync.dma_start(W1_sb[:], p_W1[:].rearrange("(c p) m -> p c m", p=128))
            nc.sync.dma_start(W2_sb[:], p_W2[:].rearrange("(c p) m -> p c m", p=128))
            nc.sync.dma_start(Wpe_sb[:], p_Wpe[:])
            nc.sync.dma_start(Wc_sb[64:128, :], p_Wc[:])
            nc.sync.dma_start(b1_sb[:], p_b1[:])
            nc.sync.dma_start(bxp_sb[:], p_bxp[:])
            nc.sync.dma_start(bc_sb[:], p_bc[:])
            nc.sync.dma_start(cwT_sb[64:128, :], p_cwT[:])
            nc.sync.dma_start(cb_sb[:], p_cb[:])
            nc.sync.dma_start(tmb_sb[:], p_tmb[:])
            nc.sync.dma_start(id_sb[:], p_id[:])
            nc.sync.dma_start(ones_sb[:], p_ones[:])
            nc.sync.dma_start(
                dinv_pp[:], p_deg[:].rearrange("o (c p) -> p (o c)", p=128)
            )
            nc.sync.dma_start(
                dinv_all[:], p_degall[:].rearrange("o (c p) -> p (o c)", p=128)
            )

            def mm2(out_ps, lhsT, rhs, start, stop):
                nc.tensor.matmul(out_ps[:, 0:512], lhsT, rhs[:, 0:512], start=start, stop=stop)
                nc.tensor.matmul(out_ps[:, 512:NPAD], lhsT, rhs[:, 512:NPAD], start=start, stop=stop)

            def rsqrt_inplace(dst, src, shape):
                # dst = 1/sqrt(src), via reciprocal + Sqrt + one Newton step
                r_ = work.tile(shape, F32, tag="nw0")
                nc.vector.reciprocal(r_[:], src)
                nc.scalar.activation(dst, r_[:], AF.Sqrt)
                iv = work.tile(shape, F32, tag="nw1")
                nc.vector.reciprocal(iv[:], dst)
                t1 = work.tile(shape, F32, tag="nw2")
                nc.vector.tensor_mul(t1[:], r_[:], iv[:])
                nc.vector.tensor_add(t1[:], t1[:], dst)
                nc.vector.tensor_scalar_mul(dst, t1[:], 0.5)

            # ---- dinv: compute in [128, BCH] layout, bounce to row layout ----
            rsqrt_inplace(dinv_pp[:], dinv_pp[:], [128, BCH])
            rsqrt_inplace(dinv_all[:], dinv_all[:], [128, NCH])
            for c in range(NCH):
                nc.vector.tensor_scalar_mul(
                    A_sb[:, c, :], A_sb[:, c, :], dinv_all[:, c:c + 1]
                )
            dsc = dramp.tile([1, NPAD], F32, tag="dsc")
            nc.sync.dma_start(
                dsc[0:1, :].rearrange("o (c p) -> p (o c)", p=128), dinv_pp[:]
            )
            nc.sync.dma_start(dinvrow[:], dsc[:])
            # dinv broadcast [128, NPAD]: rows 0:64 = dinv, rows 64:128 = 0.75*dinv
            pb = ps_c.tile([128, NPAD], F32, tag="c")
            mm2(pb, ones_sb[:], dinvrow[:], True, True)
            nc.vector.tensor_copy(dbt[0:64, :], pb[0:64, :])
            nc.vector.tensor_scalar_mul(dbt[64:128, :], pb[64:128, :], -3.0)

            # ---- initial MLP: h1 = relu(nf @ W1 + b1); x0 = h1 @ W2 + b2 ----
            h1p0 = ps_a.tile([128, NPAD], F32, tag="a")
            h1p1 = ps_s.tile([128, NPAD], F32, tag="s")
            for ci in range(4):
                nft = work.tile([128, NPAD], F32, tag="nf")
                nc.sync.dma_start(nft[:], p_nfT[ci * 128:(ci + 1) * 128, :])
                mm2(h1p0, W1_sb[:, ci, 0:128], nft[:], ci == 0, ci == 3)
                mm2(h1p1, W1_sb[:, ci, 128:256], nft[:], ci == 0, ci == 3)
            xp = ps_c.tile([64, NPAD], F32, tag="c")
            for co, h1p in enumerate((h1p0, h1p1)):
                nc.scalar.activation(h1T[:], h1p[:], AF.Relu, bias=b1_sb[:, co:co + 1])
                mm2(xp, W2_sb[:, co, :], h1T[:], co == 0, co == 1)
            nc.vector.tensor_scalar_add(vT[0:64, :], xp[:], bxp_sb[0:64, :])
            nc.vector.memset(vT[0:64, NREAL:NPAD], 0.0)

            # ---- pe0 = tanh(pos @ Wpe + bpe) (partitions 64:128) ----
            pp = ps_c.tile([128, NPAD], F32, tag="c")
            mm2(pp, Wpe_sb[:], posT_sb[:], True, True)
            nc.scalar.activation(vT[64:128, :], pp[64:128, :], AF.Tanh, bias=bxp_sb[64:128, :])
            nc.vector.memset(vT[64:128, NREAL:NPAD], 0.0)
            nc.vector.tensor_scalar_mul(hr[64:128, :], vT[64:128, :], 0.5)

            def emit_gamma_hidden(k, init):
                # gamma = temp[k]*tanh(pe @ cw[k] + cb[k]); hidden (+)= gamma * x
                g1 = ps_c.tile([1, NPAD], F32, tag="c")
                mm2(g1, cwT_sb[64:128, k:k + 1], vT[64:128, :], True, True)
                g2 = work.tile([1, NPAD], F32, tag="g2")
                nc.scalar.activation(g2[:], g1[:], AF.Tanh, bias=cb_sb[:, k:k + 1])
                gB = ps_a.tile([64, NPAD], F32, tag="a")
                mm2(gB, tmb_sb[:, k * 64:(k + 1) * 64], g2[:], True, True)
                if init:
                    nc.vector.tensor_mul(hr[0:64, :], gB[:], vT[0:64, :])
                else:
                    ht = work.tile([64, NPAD], F32, tag="ht")
                    nc.vector.tensor_mul(ht[:], gB[:], vT[0:64, :])
                    nc.vector.tensor_add(hr[0:64, :], hr[0:64, :], ht[:])

            def emit_qT():
                qp = ps_c.tile([64, NPAD], F32, tag="c")
                mm2(qp, Wc_sb[64:128, :], vT[64:128, :], True, True)
                nc.vector.tensor_scalar_add(qT_blk[0:64, :], qp[:], bc_sb[:])
                nc.sync.dma_start(qT_blk[64:128, :], qT_blk[0:64, :])

            def emit_stage_ag():
                # transpose vT -> node-major staging (scaled + raw), then AG
                stv = stpool.tile([128, BCH, 128], BF16, tag="stv")
                for cs in ([] if os.environ.get("ABL_SKIP_STAGE") == "1" else range(BCH)):
                    tp = (ps_c if cs % 2 else ps_a).tile([128, 128], F32, tag="c" if cs % 2 else "a")
                    nc.tensor.transpose(tp[:], vT[:, cs * 128:(cs + 1) * 128], id_sb[:])
                    nc.vector.tensor_copy(stv[:, cs, 0:64], tp[:, 0:64])
                    nc.vector.tensor_scalar_mul(stv[:, cs, 64:128], tp[:, 64:128], -0.25)
                agi = dramp.tile([1, AGW], BF16, tag="agi")
                nc.sync.dma_start(
                    agi[0:1, 0:VA].rearrange("o (c p f) -> p (o c) f", p=128, f=128),
                    stv[:],
                )
                nc.sync.dma_start(
                    agi[0:1, VA:AGW].rearrange("o (f t) -> f (o t)", f=64), qT_blk[0:64, :]
                )
                ago = dramp.tile([R, AGW], BF16, tag="ago", addr_space="Shared")
                if os.environ.get("ABL_SKIP_AG") == "1":
                    nc.sync.dma_start(ago[0:1, :], agi[:])
                else:
                    nc.gpsimd.collective_compute(
                        "AllGather",
                        mybir.AluOpType.bypass,
                        replica_groups=[list(range(R))],
                        ins=[agi[:].opt()],
                        outs=[ago[:].opt()],
                    )
                return ago

            emit_qT()
            ago = emit_stage_ag()
            emit_gamma_hidden(0, init=True)

            for k in range(K):
                last = k == K - 1
                # loads from gathered buffer (per-rank, qT first so S starts early)
                vst = vstp.tile([128, R, BCH, 128], BF16, tag="vst")
                qtf = qtfp.tile([128, R, NPAD], BF16, tag="qtf")
                for r_ in range(R):
                    qv = ago[r_:r_ + 1, VA:AGW].rearrange("o (f t) -> f (o t)", f=64)
                    nc.sync.dma_start(qtf[0:64, r_, :], qv)
                    nc.sync.dma_start(qtf[64:128, r_, :], qv)
                    nc.sync.dma_start(
                        vst[:, r_, :, :],
                        ago[r_:r_ + 1, 0:VA].rearrange("o (c p f) -> p (o c) f", p=128, f=128),
                    )

                # merged chunk loop: S matmul -> (prop fills ACT latency) -> corr
                # pacc rows 0:64 x_new, rows 64:128 (counts @ pe_s);
                # cacc rows 64:128 = -(1-a)*beta*(S @ pe)^T (scale staged into pe_raw)
                pacc = ps_a.tile([128, NPAD], F32, tag="a")
                cacc = ps_c.tile([128, NPAD], F32, tag="c")
                _skip_s = os.environ.get("ABL_SKIP_S") == "1"
                _skip_prop = os.environ.get("ABL_SKIP_PROP") == "1"
                for cp in range(NCH // 2):
                    c0, c1 = 2 * cp, 2 * cp + 1
                    rq0, lc0 = divmod(c0, BCH)
                    rq1, lc1 = divmod(c1, BCH)
                    if not _skip_s:
                        # two K=64 chunks concurrent in PE rows 0:64 / 64:128
                        sp = ps_s.tile([128, 2, 1024], F32, tag="s")
                        for half, (rq, lc, base) in enumerate(
                            ((rq0, lc0, 0), (rq1, lc1, 64))
                        ):
                            lhsT = qtf[base:base + 64, rq, lc * 128:(lc + 1) * 128]
                            rhs = qT_blk[base:base + 64, :]
                            nc.tensor.matmul(
                                sp[:, half, 0:512], lhsT, rhs[:, 0:512],
                                start=True, stop=True, tile_position=(base, 0),
                            )
                            nc.tensor.matmul(
                                sp[:, half, 512:NPAD], lhsT, rhs[:, 512:NPAD],
                                start=True, stop=True, tile_position=(base, 0),
                            )
                        ssb = spool.tile([128, 2, NPAD], BF16, tag="ssb")
                        nc.scalar.activation(ssb[:], sp[:, :, 0:NPAD], AF.Sigmoid)
                    for c, rq, lc, half in ((c0, rq0, lc0, 0), (c1, rq1, lc1, 1)):
                        if not _skip_prop:
                            mm2(pacc, vst[:, rq, lc, 0:128], A_sb[:, c, :], c == 0, c == NCH - 1)
                        if not _skip_s:
                            mm2(cacc, vst[:, rq, lc, 0:128], ssb[:, half, :], c == 0, c == NCH - 1)
                if _skip_s:
                    nc.tensor.matmul(cacc[:, 0:512], vst[:, 0, 0, 0:128], A_sb[:, 0, 0:512], start=True, stop=True)
                    nc.tensor.matmul(cacc[:, 512:NPAD], vst[:, 0, 0, 0:128], A_sb[:, 0, 512:NPAD], start=True, stop=True)
                if _skip_prop:
                    nc.tensor.matmul(pacc[:, 0:512], vst[:, 0, 0, 0:128], A_sb[:, 0, 0:512], start=True, stop=True)
                    nc.tensor.matmul(pacc[:, 512:NPAD], vst[:, 0, 0, 0:128], A_sb[:, 0, 512:NPAD], start=True, stop=True)

                # prop epilogues: x_new; tpo2 = 0.75*dinv*(A@pe_s) + 0.5*raw
                nc.vector.tensor_mul(vT[0:64, :], pacc[0:64, :], dbt[0:64, :])
                tpo = work.tile([128, NPAD], F32, tag="tpo")
                nc.vector.tensor_mul(tpo[64:128, :], pacc[64:128, :], dbt[64:128, :])
                nc.vector.tensor_add(tpo[64:128, :], tpo[64:128, :], hr[64:128, :])

                # pe update: pe = tanh(tpo2 + cacc)
                cs_t = work.tile([128, NPAD], F32, tag="cs")
                nc.vector.tensor_add(cs_t[64:128, :], cacc[64:128, :], tpo[64:128, :])
                nc.scalar.activation(vT[64:128, :], cs_t[64:128, :], AF.Tanh)
                nc.vector.memset(vT[64:128, NREAL:NPAD], 0.0)

                if not last:
                    emit_qT()
                    ago = emit_stage_ag()
                emit_gamma_hidden(k + 1, init=False)

            nc.sync.dma_start(out_h[:], hr[0:64, :])
            nc.sync.dma_start(out_pe[:], vT[64:128, :])

    nc.finalize()
    return nc


def _preprocess(node_feat, pos_enc, edge_index, W1, b1, W2, b2, Wpe, bpe, Wc, bc,
                temp, coeff_w, coeff_b):
    """Host-side integer index preprocessing + per-rank input shards."""
    nf = np.asarray(node_feat, np.float32)
    pos = np.asarray(pos_enc, np.float32)
    ei = np.asarray(edge_index)
    row = ei[0].astype(np.int64)
    col = ei[1].astype(np.int64)
    n = nf.shape[0]

    deg = (np.bincount(col, minlength=n) + 1).astype(np.float32)  # int counts
    deg_pad = np.ones((R, NPAD), np.float32)
    ridx, lidx = np.divmod(np.arange(n), NREAL)
    deg_pad[ridx, lidx] = deg
    deg_all = np.ascontiguousarray(deg_pad.reshape(1, NG))

    counts = np.zeros((R, NG, NPAD), np.float32)
    grow = (row // NREAL) * NPAD + (row % NREAL)
    np.add.at(counts, (col // NREAL, grow, col % NREAL), 1.0)
    gv = ridx * NPAD + lidx
    np.add.at(counts, (ridx, gv, lidx), 1.0)
    counts_bf = counts.astype(ml_dtypes.bfloat16)

    def pad_T(x, dim):
        blocks = []
        for r in range(R):
            blk = np.zeros((NPAD, dim), np.float32)
            blk[:NREAL] = x[r * NREAL:(r + 1) * NREAL]
            blocks.append(np.ascontiguousarray(blk.T))
        return blocks

    nfT_blocks = pad_T(nf, 512)
    posT_blocks = pad_T(pos, 32)

    Wpe_pad = np.zeros((32, 128), np.float32)
    Wpe_pad[:, 64:128] = np.asarray(Wpe, np.float32)
    bias_xpe = np.concatenate(
        [np.asarray(b2, np.float32), np.asarray(bpe, np.float32)]
    )[:, None]

    common = {
        "W1": np.asarray(W1, np.float32),
        "W2": np.asarray(W2, np.float32),
        "Wpe": Wpe_pad,
        "Wc": np.asarray(Wc, np.float32),
        "b1": np.ascontiguousarray(np.asarray(b1, np.float32).reshape(2, 128).T),
        "bias_xpe": bias_xpe,
        "bc": np.asarray(bc, np.float32)[:, None],
        "cwT": np.ascontiguousarray(np.asarray(coeff_w, np.float32).T),
        "cb": np.asarray(coeff_b, np.float32)[None, :],
        "tmb": np.ascontiguousarray(np.repeat(np.asarray(temp, np.float32), 64)[None, :]),
        "ident": np.eye(128, dtype=np.float32),
        "ones128": np.ones((1, 128), np.float32),
    }
    in_maps = []
    for r in range(R):
        m = dict(common)
        m["nfT"] = nfT_blocks[r]
        m["posT"] = posT_blocks[r]
        m["cnt"] = counts_bf[r]
        m["deg"] = deg_pad[r][None, :]
        m["deg_all"] = deg_all
        in_maps.append(m)
    return in_maps


def kernel(**inputs):
    global _NC, LAST_EXEC_NS
    from concourse.bass_utils import run_bass_kernel_spmd

    in_maps = _preprocess(**inputs)
    if _NC is None:
        _NC = _build()
    trace = os.environ.get("KERNEL_TRACE", "0") == "1"
    res = run_bass_kernel_spmd(_NC, in_maps, list(range(R)), trace=trace)
    LAST_EXEC_NS = res.exec_time_ns
    hidden = np.concatenate(
        [res.results[r]["out_h"].T[:NREAL] for r in range(R)], axis=0
    ).astype(np.float32)
    pe = np.concatenate(
        [res.results[r]["out_pe"].T[:NREAL] for r in range(R)], axis=0
    ).astype(np.float32)
    return hidden, pe


# revision 13
# speedup vs baseline: 1.1399x; 1.1399x over previous
"""Trainium2 Bass kernel for nn_DSF_GPR_I (gnn_message_passing), 8 NeuronCores.

Strategy (per sharding_hint: shard nodes across devices; N x N sigmoid
correlation row-block-parallel with all-gathered state):
  - 6000 nodes -> 8 ranks x 750, padded to 768 (=6*128) per rank.
  - gcn_norm scatter-add propagation A@v is computed as a dense matmul against
    the per-rank column block of the adjacency COUNT matrix (integer edge
    multiplicities, exact in bf16, SBUF-resident), with D^-1/2 applied on
    device: source side folded into the all-gathered vectors, target side via
    a broadcast multiply.
  - Per iteration: one AllGather (bf16) of [x_s | pe_s | pe_raw] (node-major)
    + qT (feature-major); prop + the row-block S = sigmoid(q_blk @ q^T) tiles
    (48 chunks of 128 columns) + corr accumulation all run from SBUF.
  - Host does integer-only index preprocessing (bincount degrees, dense count
    matrix, transposes/padding). All float math runs on device.

Partition layout: x-state on partitions 0:64, pe-state on partitions 64:128
(DVE/ACT lanes are partition-hardwired, so elementwise chains stay on a
consistent base; stationary matmul operands that pair with pe-state are loaded
at base 64).
"""
import os
import numpy as np
import ml_dtypes

R = 8
NREAL = 750
NPAD = 768
NG = R * NPAD           # 6144
NCH = NG // 128         # 48 chunks
BCH = NPAD // 128       # 6 chunks per rank block
K = 10
VA = NPAD * 128         # 98304  (node-major region: [x_raw | -0.25*pe])
QTW = 64 * NPAD         # 49152   (feature-major qT region)
AGW = VA + QTW          # 196608

LAST_EXEC_NS = None
_NC = None


def _build():
    import concourse.bacc as bacc
    import concourse.mybir as mybir
    import concourse.tile as tile

    F32 = mybir.dt.float32
    BF16 = mybir.dt.bfloat16
    AF = mybir.ActivationFunctionType

    nc = bacc.Bacc("TRN2", target_bir_lowering=False, debug=False, num_devices=R)

    # ---- I/O ----
    p_nfT = nc.declare_dram_parameter("nfT", [512, NPAD], F32, isOutput=False)
    p_posT = nc.declare_dram_parameter("posT", [32, NPAD], F32, isOutput=False)
    p_cnt = nc.declare_dram_parameter("cnt", [NG, NPAD], BF16, isOutput=False)
    p_deg = nc.declare_dram_parameter("deg", [1, NPAD], F32, isOutput=False)
    p_degall = nc.declare_dram_parameter("deg_all", [1, NG], F32, isOutput=False)
    p_W1 = nc.declare_dram_parameter("W1", [512, 256], F32, isOutput=False)
    p_W2 = nc.declare_dram_parameter("W2", [256, 64], F32, isOutput=False)
    p_Wpe = nc.declare_dram_parameter("Wpe", [32, 128], F32, isOutput=False)  # [0|Wpe]
    p_Wc = nc.declare_dram_parameter("Wc", [64, 64], F32, isOutput=False)
    p_b1 = nc.declare_dram_parameter("b1", [128, 2], F32, isOutput=False)
    p_bxp = nc.declare_dram_parameter("bias_xpe", [128, 1], F32, isOutput=False)  # [b2; bpe]
    p_bc = nc.declare_dram_parameter("bc", [64, 1], F32, isOutput=False)
    p_cwT = nc.declare_dram_parameter("cwT", [64, K + 1], F32, isOutput=False)
    p_cb = nc.declare_dram_parameter("cb", [1, K + 1], F32, isOutput=False)
    p_tmb = nc.declare_dram_parameter("tmb", [1, (K + 1) * 64], F32, isOutput=False)
    p_id = nc.declare_dram_parameter("ident", [128, 128], F32, isOutput=False)
    p_ones = nc.declare_dram_parameter("ones128", [1, 128], F32, isOutput=False)
    out_h = nc.declare_dram_parameter("out_h", [64, NPAD], F32, isOutput=True)
    out_pe = nc.declare_dram_parameter("out_pe", [64, NPAD], F32, isOutput=True)

    with tile.TileContext(nc) as tc:
        with (
            tc.tile_pool(name="pers", bufs=1) as pers,
            tc.tile_pool(name="vstp", bufs=1) as vstp,
            tc.tile_pool(name="qtfp", bufs=1) as qtfp,
            tc.tile_pool(name="spool", bufs=3) as spool,
            tc.tile_pool(name="stpool", bufs=2) as stpool,
            tc.tile_pool(name="work", bufs=2) as work,
            tc.tile_pool(name="dramp", bufs=2, space="DRAM") as dramp,
            tc.tile_pool(name="ps_a", bufs=1, space="PSUM") as ps_a,
            tc.tile_pool(name="ps_s", bufs=2, space="PSUM") as ps_s,
            tc.tile_pool(name="ps_c", bufs=1, space="PSUM") as ps_c,
        ):
            # ---- persistent SBUF ----
            A_sb = pers.tile([128, NCH, NPAD], BF16)
            vT = pers.tile([128, NPAD], F32)     # rows 0:64 x, 64:128 pe
            hr = pers.tile([128, NPAD], F32)     # rows 0:64 hidden, 64:128 0.5*pe0
            qT_blk = pers.tile([128, NPAD], BF16)
            dbt = pers.tile([128, NPAD], F32)    # rows 0:64 dinv, 64:128 0.75*dinv
            dinv_pp = pers.tile([128, BCH], F32)
            dinv_all = pers.tile([128, NCH], F32)
            W1_sb = pers.tile([128, 4, 256], F32)
            W2_sb = pers.tile([128, 2, 64], F32)
            Wpe_sb = pers.tile([32, 128], F32)
            Wc_sb = pers.tile([128, 64], F32)    # rows 64:128 hold Wc
            b1_sb = pers.tile([128, 2], F32)
            bxp_sb = pers.tile([128, 1], F32)
            bc_sb = pers.tile([64, 1], F32)
            cwT_sb = pers.tile([128, K + 1], F32)  # rows 64:128 hold coeff_w^T
            cb_sb = pers.tile([1, K + 1], F32)
            tmb_sb = pers.tile([1, (K + 1) * 64], F32)
            id_sb = pers.tile([128, 128], F32)
            ones_sb = pers.tile([1, 128], F32)
            posT_sb = pers.tile([32, NPAD], F32)
            h1T = pers.tile([128, NPAD], F32)
            dinvrow = pers.tile([1, NPAD], F32)

            # ---- input loads ----
            nc.sync.dma_start(A_sb[:], p_cnt[:].rearrange("(c p) t -> p c t", p=128))
            nc.sync.dma_start(posT_sb[:], p_posT[:])
            nc.sync.dma_start(W1_sb[:], p_W1[:].rearrange("(c p) m -> p c m", p=128))
            nc.sync.dma_start(W2_sb[:], p_W2[:].rearrange("(c p) m -> p c m", p=128))
            nc.sync.dma_start(Wpe_sb[:], p_Wpe[:])
            nc.sync.dma_start(Wc_sb[64:128, :], p_Wc[:])
            nc.sync.dma_start(b1_sb[:], p_b1[:])
            nc.sync.dma_start(bxp_sb[:], p_bxp[:])
            nc.sync.dma_start(bc_sb[:], p_bc[:])
            nc.sync.dma_start(cwT_sb[64:128, :], p_cwT[:])
            nc.sync.dma_start(cb_sb[:], p_cb[:])
            nc.sync.dma_start(tmb_sb[:], p_tmb[:])
            nc.sync.dma_start(id_sb[:], p_id[:])
            nc.sync.dma_start(ones_sb[:], p_ones[:])
            nc.sync.dma_start(
                dinv_pp[:], p_deg[:].rearrange("o (c p) -> p (o c)", p=128)
            )
            nc.sync.dma_start(
                dinv_all[:], p_degall[:].rearrange("o (c p) -> p (o c)", p=128)
            )

            def mm2(out_ps, lhsT, rhs, start, stop):
                nc.tensor.matmul(out_ps[:, 0:512], lhsT, rhs[:, 0:512], start=start, stop=stop)
                nc.tensor.matmul(out_ps[:, 512:NPAD], lhsT, rhs[:, 512:NPAD], start=start, stop=stop)

            def rsqrt_inplace(dst, src, shape):
                # dst = 1/sqrt(src), via reciprocal + Sqrt + one Newton step
                r_ = work.tile(shape, F32, tag="nw0")
                nc.vector.reciprocal(r_[:], src)
                nc.scalar.activation(dst, r_[:], AF.Sqrt)
                iv = work.tile(shape, F32, tag="nw1")
                nc.vector.reciprocal(iv[:], dst)
                t1 = work.tile(shape, F32, tag="nw2")
                nc.vector.tensor_mul(t1[:], r_[:], iv[:])
                nc.vector.tensor_add(t1[:], t1[:], dst)
                nc.vector.tensor_scalar_mul(dst, t1[:], 0.5)

            # ---- dinv: compute in [128, BCH] layout, bounce to row layout ----
            rsqrt_inplace(dinv_pp[:], dinv_pp[:], [128, BCH])
            rsqrt_inplace(dinv_all[:], dinv_all[:], [128, NCH])
            for c in range(NCH):
                nc.vector.tensor_scalar_mul(
                    A_sb[:, c, :], A_sb[:, c, :], dinv_all[:, c:c + 1]
                )
            dsc = dramp.tile([1, NPAD], F32, tag="dsc")
            nc.sync.dma_start(
                dsc[0:1, :].rearrange("o (c p) -> p (o c)", p=128), dinv_pp[:]
            )
            nc.sync.dma_start(dinvrow[:], dsc[:])
            # dinv broadcast [128, NPAD]: rows 0:64 = dinv, rows 64:128 = 0.75*dinv
            pb = ps_c.tile([128, NPAD], F32, tag="c")
            mm2(pb, ones_sb[:], dinvrow[:], True, True)
            nc.vector.tensor_copy(dbt[0:64, :], pb[0:64, :])
            nc.vector.tensor_scalar_mul(dbt[64:128, :], pb[64:128, :], -3.0)

            # ---- initial MLP: h1 = relu(nf @ W1 + b1); x0 = h1 @ W2 + b2 ----
            h1p0 = ps_a.tile([128, NPAD], F32, tag="a")
            h1p1 = ps_s.tile([128, NPAD], F32, tag="s")
            for ci in range(4):
                nft = work.tile([128, NPAD], F32, tag="nf")
                nc.sync.dma_start(nft[:], p_nfT[ci * 128:(ci + 1) * 128, :])
                mm2(h1p0, W1_sb[:, ci, 0:128], nft[:], ci == 0, ci == 3)
                mm2(h1p1, W1_sb[:, ci, 128:256], nft[:], ci == 0, ci == 3)
            xp = ps_c.tile([64, NPAD], F32, tag="c")
            for co, h1p in enumerate((h1p0, h1p1)):
                nc.scalar.activation(h1T[:], h1p[:], AF.Relu, bias=b1_sb[:, co:co + 1])
                mm2(xp, W2_sb[:, co, :], h1T[:], co == 0, co == 1)
            nc.vector.tensor_scalar_add(vT[0:64, :], xp[:], bxp_sb[0:64, :])
            nc.vector.memset(vT[0:64, NREAL:NPAD], 0.0)

            # ---- pe0 = tanh(pos @ Wpe + bpe) (partitions 64:128) ----
            pp = ps_c.tile([128, NPAD], F32, tag="c")
            mm2(pp, Wpe_sb[:], posT_sb[:], True, True)
            nc.scalar.activation(vT[64:128, :], pp[64:128, :], AF.Tanh, bias=bxp_sb[64:128, :])
            nc.vector.memset(vT[64:128, NREAL:NPAD], 0.0)
            nc.vector.tensor_scalar_mul(hr[64:128, :], vT[64:128, :], 0.5)

            def emit_gamma_hidden(k, init):
                # gamma = temp[k]*tanh(pe @ cw[k] + cb[k]); hidden (+)= gamma * x
                g1 = ps_c.tile([1, NPAD], F32, tag="c")
                mm2(g1, cwT_sb[64:128, k:k + 1], vT[64:128, :], True, True)
                g2 = work.tile([1, NPAD], F32, tag="g2")
                nc.scalar.activation(g2[:], g1[:], AF.Tanh, bias=cb_sb[:, k:k + 1])
                gB = ps_a.tile([64, NPAD], F32, tag="a")
                mm2(gB, tmb_sb[:, k * 64:(k + 1) * 64], g2[:], True, True)
                if init:
                    nc.vector.tensor_mul(hr[0:64, :], gB[:], vT[0:64, :])
                else:
                    ht = work.tile([64, NPAD], F32, tag="ht")
                    nc.vector.tensor_mul(ht[:], gB[:], vT[0:64, :])
                    nc.vector.tensor_add(hr[0:64, :], hr[0:64, :], ht[:])

            def emit_qT():
                qp = ps_c.tile([64, NPAD], F32, tag="c")
                mm2(qp, Wc_sb[64:128, :], vT[64:128, :], True, True)
                nc.vector.tensor_scalar_add(qT_blk[0:64, :], qp[:], bc_sb[:])
                nc.sync.dma_start(qT_blk[64:128, :], qT_blk[0:64, :])

            def emit_stage_ag():
                # transpose vT -> node-major staging (scaled + raw), then AG
                stv = stpool.tile([128, BCH, 128], BF16, tag="stv")
                for cs in ([] if os.environ.get("ABL_SKIP_STAGE") == "1" else range(BCH)):
                    tp = (ps_c if cs % 2 else ps_a).tile([128, 128], F32, tag="c" if cs % 2 else "a")
                    nc.tensor.transpose(tp[:], vT[:, cs * 128:(cs + 1) * 128], id_sb[:])
                    nc.vector.tensor_copy(stv[:, cs, 0:64], tp[:, 0:64])
                    nc.vector.tensor_scalar_mul(stv[:, cs, 64:128], tp[:, 64:128], -0.25)
                agi = dramp.tile([1, AGW], BF16, tag="agi")
                nc.sync.dma_start(
                    agi[0:1, 0:VA].rearrange("o (c p f) -> p (o c) f", p=128, f=128),
                    stv[:],
                )
                nc.sync.dma_start(
                    agi[0:1, VA:AGW].rearrange("o (f t) -> f (o t)", f=64), qT_blk[0:64, :]
                )
                ago = dramp.tile([R, AGW], BF16, tag="ago", addr_space="Shared")
                if os.environ.get("ABL_SKIP_AG") == "1":
                    nc.sync.dma_start(ago[0:1, :], agi[:])
                else:
                    nc.gpsimd.collective_compute(
                        "AllGather",
                        mybir.AluOpType.bypass,
                        replica_groups=[list(range(R))],
                        ins=[agi[:].opt()],
                        outs=[ago[:].opt()],
                    )
                return ago

            emit_qT()
            ago = emit_stage_ag()
            emit_gamma_hidden(0, init=True)

            for k in range(K):
                last = k == K - 1
                # loads from gathered buffer (per-rank, qT first so S starts early)
                vst = vstp.tile([128, R, BCH, 128], BF16, tag="vst")
                qtf = qtfp.tile([128, R, NPAD], BF16, tag="qtf")
                for r_ in range(R):
                    qv = ago[r_:r_ + 1, VA:AGW].rearrange("o (f t) -> f (o t)", f=64)
                    nc.sync.dma_start(qtf[0:64, r_, :], qv)
                    nc.sync.dma_start(qtf[64:128, r_, :], qv)
                    nc.sync.dma_start(
                        vst[:, r_, :, :],
                        ago[r_:r_ + 1, 0:VA].rearrange("o (c p f) -> p (o c) f", p=128, f=128),
                    )

                # merged chunk loop: S matmul -> (prop fills ACT latency) -> corr
                # pacc rows 0:64 x_new, rows 64:128 (counts @ pe_s);
                # cacc rows 64:128 = -(1-a)*beta*(S @ pe)^T (scale staged into pe_raw)
                pacc = ps_a.tile([128, NPAD], F32, tag="a")
                cacc = ps_c.tile([128, NPAD], F32, tag="c")
                _skip_s = os.environ.get("ABL_SKIP_S") == "1"
                _skip_prop = os.environ.get("ABL_SKIP_PROP") == "1"
                for cp in range(NCH // 2):
                    c0, c1 = 2 * cp, 2 * cp + 1
                    rq0, lc0 = divmod(c0, BCH)
                    rq1, lc1 = divmod(c1, BCH)
                    sps = []
                    if not _skip_s:
                        # two K=64 chunks concurrent in PE rows 0:64 / 64:128
                        for rq, lc, base in ((rq0, lc0, 0), (rq1, lc1, 64)):
                            sp = ps_s.tile([128, NPAD], F32, tag="s")
                            lhsT = qtf[base:base + 64, rq, lc * 128:(lc + 1) * 128]
                            rhs = qT_blk[base:base + 64, :]
                            nc.tensor.matmul(
                                sp[:, 0:512], lhsT, rhs[:, 0:512],
                                start=True, stop=True, tile_position=(base, 0),
                            )
                            nc.tensor.matmul(
                                sp[:, 512:NPAD], lhsT, rhs[:, 512:NPAD],
                                start=True, stop=True, tile_position=(base, 0),
                            )
                            sps.append(sp)
                    for c, rq, lc, half in ((c0, rq0, lc0, 0), (c1, rq1, lc1, 1)):
                        if not _skip_s:
                            ssb = spool.tile([128, NPAD], BF16, tag="ssb")
                            nc.scalar.activation(ssb[:], sps[half][:], AF.Sigmoid)
                        if not _skip_prop:
                            mm2(pacc, vst[:, rq, lc, 0:128], A_sb[:, c, :], c == 0, c == NCH - 1)
                        if not _skip_s:
                            mm2(cacc, vst[:, rq, lc, 0:128], ssb[:], c == 0, c == NCH - 1)
                if _skip_s:
                    nc.tensor.matmul(cacc[:, 0:512], vst[:, 0, 0, 0:128], A_sb[:, 0, 0:512], start=True, stop=True)
                    nc.tensor.matmul(cacc[:, 512:NPAD], vst[:, 0, 0, 0:128], A_sb[:, 0, 512:NPAD], start=True, stop=True)
                if _skip_prop:
                    nc.tensor.matmul(pacc[:, 0:512], vst[:, 0, 0, 0:128], A_sb[:, 0, 0:512], start=True, stop=True)
                    nc.tensor.matmul(pacc[:, 512:NPAD], vst[:, 0, 0, 0:128], A_sb[:, 0, 512:NPAD], start=True, stop=True)

                # prop epilogues: x_new; tpo2 = 0.75*dinv*(A@pe_s) + 0.5*raw
                nc.vector.tensor_mul(vT[0:64, :], pacc[0:64, :], dbt[0:64, :])
                tpo = work.tile([128, NPAD], F32, tag="tpo")
                nc.vector.tensor_mul(tpo[64:128, :], pacc[64:128, :], dbt[64:128, :])
                nc.vector.tensor_add(tpo[64:128, :], tpo[64:128, :], hr[64:128, :])

                # pe update: pe = tanh(tpo2 + cacc)
                cs_t = work.tile([128, NPAD], F32, tag="cs")
                nc.vector.tensor_add(cs_t[64:128, :], cacc[64:128, :], tpo[64:128, :])
                nc.scalar.activation(vT[64:128, :], cs_t[64:128, :], AF.Tanh)
                nc.vector.memset(vT[64:128, NREAL:NPAD], 0.0)

                if not last:
                    emit_qT()
                    ago = emit_stage_ag()
                emit_gamma_hidden(k + 1, init=False)

            nc.sync.dma_start(out_h[:], hr[0:64, :])
            nc.sync.dma_start(out_pe[:], vT[64:128, :])

    nc.finalize()
    return nc


def _preprocess(node_feat, pos_enc, edge_index, W1, b1, W2, b2, Wpe, bpe, Wc, bc,
                temp, coeff_w, coeff_b):
    """Host-side integer index preprocessing + per-rank input shards."""
    nf = np.asarray(node_feat, np.float32)
    pos = np.asarray(pos_enc, np.float32)
    ei = np.asarray(edge_index)
    row = ei[0].astype(np.int64)
    col = ei[1].astype(np.int64)
    n = nf.shape[0]

    deg = (np.bincount(col, minlength=n) + 1).astype(np.float32)  # int counts
    deg_pad = np.ones((R, NPAD), np.float32)
    ridx, lidx = np.divmod(np.arange(n), NREAL)
    deg_pad[ridx, lidx] = deg
    deg_all = np.ascontiguousarray(deg_pad.reshape(1, NG))

    counts = np.zeros((R, NG, NPAD), np.float32)
    grow = (row // NREAL) * NPAD + (row % NREAL)
    np.add.at(counts, (col // NREAL, grow, col % NREAL), 1.0)
    gv = ridx * NPAD + lidx
    np.add.at(counts, (ridx, gv, lidx), 1.0)
    counts_bf = counts.astype(ml_dtypes.bfloat16)

    def pad_T(x, dim):
        blocks = []
        for r in range(R):
            blk = np.zeros((NPAD, dim), np.float32)
            blk[:NREAL] = x[r * NREAL:(r + 1) * NREAL]
            blocks.append(np.ascontiguousarray(blk.T))
        return blocks

    nfT_blocks = pad_T(nf, 512)
    posT_blocks = pad_T(pos, 32)

    Wpe_pad = np.zeros((32, 128), np.float32)
    Wpe_pad[:, 64:128] = np.asarray(Wpe, np.float32)
    bias_xpe = np.concatenate(
        [np.asarray(b2, np.float32), np.asarray(bpe, np.float32)]
    )[:, None]

    common = {
        "W1": np.asarray(W1, np.float32),
        "W2": np.asarray(W2, np.float32),
        "Wpe": Wpe_pad,
        "Wc": np.asarray(Wc, np.float32),
        "b1": np.ascontiguousarray(np.asarray(b1, np.float32).reshape(2, 128).T),
        "bias_xpe": bias_xpe,
        "bc": np.asarray(bc, np.float32)[:, None],
        "cwT": np.ascontiguousarray(np.asarray(coeff_w, np.float32).T),
        "cb": np.asarray(coeff_b, np.float32)[None, :],
        "tmb": np.ascontiguousarray(np.repeat(np.asarray(temp, np.float32), 64)[None, :]),
        "ident": np.eye(128, dtype=np.float32),
        "ones128": np.ones((1, 128), np.float32),
    }
    in_maps = []
    for r in range(R):
        m = dict(common)
        m["nfT"] = nfT_blocks[r]
        m["posT"] = posT_blocks[r]
        m["cnt"] = counts_bf[r]
        m["deg"] = deg_pad[r][None, :]
        m["deg_all"] = deg_all
        in_maps.append(m)
    return in_maps


def kernel(**inputs):
    global _NC, LAST_EXEC_NS
    from concourse.bass_utils import run_bass_kernel_spmd

    in_maps = _preprocess(**inputs)
    if _NC is None:
        _NC = _build()
    trace = os.environ.get("KERNEL_TRACE", "0") == "1"
    res = run_bass_kernel_spmd(_NC, in_maps, list(range(R)), trace=trace)
    LAST_EXEC_NS = res.exec_time_ns
    hidden = np.concatenate(
        [res.results[r]["out_h"].T[:NREAL] for r in range(R)], axis=0
    ).astype(np.float32)
    pe = np.concatenate(
        [res.results[r]["out_pe"].T[:NREAL] for r in range(R)], axis=0
    ).astype(np.float32)
    return hidden, pe
